# revision 11
# baseline (speedup 1.0000x reference)
"""Elman RNN (DummyRNN) Trainium2 Bass kernel.

Math: h_t = tanh(x_t @ Ww.T + h_{t-1} @ Uw.T + (Wb + Ub + b)), t = 0..T-1
Output: concat over t of h_t  -> [T*B, D_OUT]

Strategy (data-parallel over batch, 8 cores, B_local = 8):
  Phase A: Z = X_local @ Ww.T + bias, batched over all T*B_local rows
           (dense full-array matmuls), stored to internal DRAM in a
           column-group layout matching phase B.
  Phase B: sequential recurrence with PE column-tiling: the output
           feature dim is split into 4 column groups of the 128x128
           array (tile_position=(0, 32j)); the 4 groups' U-stream
           matmuls run concurrently, 4x faster than a single stream.
           Group j accumulates pre-activations for features
           [512j, 512j+512) at PSUM partitions [32j, 32j+8).
           z_t is added into PSUM by an identity-stationary matmul,
           tanh reads PSUM directly on ScalarE, and per-step h
           transposes (PE) produce the next step's stationary tiles.
  All matmul operands bf16 (fp32 PSUM accumulate): the recurrence is
  contractive so per-step rounding does not amplify (measured ~4e-3 rel).
  Host pre-transposes X/Ww/Uw so no on-chip input transposes are needed.
"""

import sys

for _p in ("/opt/trn_rl_repo",):
    if _p not in sys.path:
        sys.path.insert(0, _p)

import numpy as np
import ml_dtypes

import concourse.bass as bass
import concourse.bacc as bacc
import concourse.tile as tile
from concourse import mybir
from concourse.bass_utils import run_bass_kernel_spmd

BF16 = ml_dtypes.bfloat16

T, B, DIN, DOUT = 512, 64, 1024, 2048
NCORES = 8
BL = B // NCORES          # batch rows per core
P = 128                   # partitions
NG = 4                    # column groups
GW = DOUT // NG           # features per group (512)
KCH = DOUT // P           # contraction chunks for U (16)
KCH_W = DIN // P          # contraction chunks for Ww (8)
KPG = KCH // NG           # k-windows per group (4)


def _build_nc(t_steps: int, repeats: int = 1) -> bass.Bass:
    nc = bacc.Bacc()
    dt = mybir.dt
    TANH = mybir.ActivationFunctionType.Tanh

    rows = t_steps * BL
    xT = nc.dram_tensor("xt", [DIN, rows], dt.bfloat16, kind="ExternalInput")
    wwT = nc.dram_tensor("wwt", [DIN, DOUT], dt.bfloat16, kind="ExternalInput")
    uT = nc.dram_tensor("ut", [DOUT, DOUT], dt.bfloat16, kind="ExternalInput")
    biasr = nc.dram_tensor("biasr", [1, DOUT], dt.bfloat16, kind="ExternalInput")
    # identity blocks at partition offsets 32j (for transposes + z-adds)
    identb = nc.dram_tensor("identb", [P, BL], dt.bfloat16, kind="ExternalInput")
    ones = nc.dram_tensor("ones", [1, P], dt.bfloat16, kind="ExternalInput")
    ys = nc.dram_tensor("ys", [rows, DOUT], dt.bfloat16, kind="ExternalOutput")

    n_mtiles = rows // P
    assert rows % P == 0
    tpb = P // NG             # 32: partitions per group slot

    with tile.TileContext(nc) as tc:
        with (
            tc.tile_pool(name="const", bufs=1) as const,
            tc.tile_pool(name="dram", bufs=1, space="DRAM") as dram,
        ):
            identb_sb = const.tile([P, BL], dt.bfloat16)
            nc.sync.dma_start(out=identb_sb, in_=identb[:, :])
            # z in column-group layout: [t, 32j+b, m] = z[t, b, 512j+m]
            zbuf = dram.tile([t_steps, P, GW], dt.bfloat16)

            # ---- Phase A: Z = X @ Ww.T + bias ----
            with (
                tc.tile_pool(name="aweights", bufs=1) as aweights,
                tc.tile_pool(name="xt_pool", bufs=2) as xt_pool,
                tc.tile_pool(name="zout", bufs=4) as zout,
                tc.tile_pool(name="psumA", bufs=4, space="PSUM") as psumA,
            ):
                wwT_sb = aweights.tile([P, KCH_W, DOUT], dt.bfloat16)
                for k in range(KCH_W):
                    nc.sync.dma_start(
                        out=wwT_sb[:, k, :], in_=wwT[k * P:(k + 1) * P, :]
                    )
                bias_sb = aweights.tile([1, DOUT], dt.bfloat16)
                nc.sync.dma_start(out=bias_sb, in_=biasr[:, :])
                ones_sb = aweights.tile([1, P], dt.bfloat16)
                nc.sync.dma_start(out=ones_sb, in_=ones[:, :])

                # m-tile covers timesteps [m*16, m*16+16)
                t_per_mtile = P // BL  # 16
                for m in range(n_mtiles):
                    xt_tiles = []
                    for k in range(KCH_W):
                        xt_t = xt_pool.tile([P, P], dt.bfloat16, tag=f"xt{k}")
                        nc.sync.dma_start(
                            out=xt_t,
                            in_=xT[k * P:(k + 1) * P, m * P:(m + 1) * P],
                        )
                        xt_tiles.append(xt_t)
                    for g in range(NG):
                        sl = slice(g * GW, (g + 1) * GW)
                        ps = psumA.tile([P, GW], dt.float32, tag="psA")
                        for k in range(KCH_W):
                            nc.tensor.matmul(
                                ps,
                                xt_tiles[k],
                                wwT_sb[:, k, sl],
                                start=(k == 0),
                                stop=False,
                            )
                        nc.tensor.matmul(
                            ps, ones_sb, bias_sb[:, sl], start=False, stop=True,
                        )
                        zt = zout.tile([P, GW], dt.bfloat16, tag="zo")
                        nc.scalar.copy(zt, ps)
                        # rows p = 8*tl + b of this m-tile go to
                        # zbuf[m*16 + tl, 32g + b, :]
                        dst = bass.AP(
                            tensor=zbuf.tensor,
                            offset=zbuf.offset
                            + (m * t_per_mtile) * (P * GW)
                            + (g * tpb) * GW,
                            ap=[[P * GW, t_per_mtile], [GW, BL], [1, GW]],
                        )
                        nc.sync.dma_start(out=dst, in_=zt)

            # ---- Phase B: recurrence ----
            with (
                tc.tile_pool(name="u_res", bufs=1) as u_res,
                tc.tile_pool(name="hT", bufs=2) as hT_pool,
                tc.tile_pool(name="hbuf", bufs=3) as hbuf,
                tc.tile_pool(name="zin", bufs=4) as zin,
                tc.tile_pool(name="psumB", bufs=1, space="PSUM") as psumB,
                tc.tile_pool(name="psumT", bufs=1, space="PSUM") as psumT,
            ):
                uT_sb = u_res.tile([P, KCH, DOUT], dt.bfloat16)
                for k in range(KCH):
                    nc.sync.dma_start(
                        out=uT_sb[:, k, :], in_=uT[k * P:(k + 1) * P, :]
                    )

                for _rep in range(repeats):
                  hT_cur = None  # t=0: h_{-1} = 0 -> no U matmuls
                  for t in range(t_steps):
                    zt = zin.tile([P, GW], dt.bfloat16, tag="zt")
                    nc.sync.dma_start(out=zt, in_=zbuf[t])
                    ps = psumB.tile([P, DOUT], dt.float32, tag="psB")
                    for g in range(NG):
                        gsl = (slice(g * tpb, g * tpb + BL),
                               slice(g * GW, (g + 1) * GW))
                        if hT_cur is not None:
                            for k in range(KCH):
                                nc.tensor.matmul(
                                    ps[gsl],
                                    hT_cur[k],
                                    uT_sb[:, k, g * GW:(g + 1) * GW],
                                    start=(k == 0),
                                    stop=False,
                                    tile_position=(0, g * tpb),
                                )
                        # z-add: identity-stationary matmul injects z_t
                        nc.tensor.matmul(
                            ps[gsl],
                            identb_sb[g * tpb:g * tpb + BL, :],
                            zt[g * tpb:g * tpb + BL, :],
                            start=(hT_cur is None),
                            stop=True,
                            tile_position=(g * tpb, g * tpb),
                        )
                    h = hbuf.tile([P, DOUT], dt.bfloat16, tag="h")
                    nc.scalar.activation(h, ps, TANH)
                    for g in range(NG):
                        nc.sync.dma_start(
                            out=ys[t * BL:(t + 1) * BL, g * GW:(g + 1) * GW],
                            in_=h[g * tpb:g * tpb + BL, g * GW:(g + 1) * GW],
                        )
                    if t == t_steps - 1:
                        break
                    hT_next = []
                    for k in range(KCH):
                        g = k // KPG
                        pst = psumT.tile(
                            [P, BL], dt.bfloat16,
                            tag=f"pst{k % 4}", name=f"pst{k % 4}",
                        )
                        nc.tensor.transpose(
                            pst,
                            h[g * tpb:g * tpb + BL, k * P:(k + 1) * P],
                            identb_sb[g * tpb:g * tpb + BL, :],
                            tile_position=(g * tpb, 0),
                        )
                        ht = hT_pool.tile(
                            [P, BL], dt.bfloat16, tag=f"hT{k}", name=f"hT{k}"
                        )
                        nc.vector.tensor_copy(ht, pst)
                        hT_next.append(ht)
                    hT_cur = hT_next

    nc.compile()
    return nc


_NC_CACHE: dict[int, bass.Bass] = {}
LAST_EXEC_NS = None
LAST_PROFILE = None


def _prep_inputs(x, Ww, Uw, bias, t_steps):
    wwT = np.ascontiguousarray(Ww.T).astype(BF16)          # [DIN, DOUT]
    uT = np.ascontiguousarray(Uw.T).astype(BF16)           # [DOUT, DOUT]
    biasr = bias.reshape(1, DOUT).astype(BF16)
    identb = np.zeros((P, BL), dtype=BF16)
    for j in range(NG):
        for c in range(BL):
            identb[j * (P // NG) + c, c] = 1
    ones = np.ones((1, P), dtype=BF16)

    in_maps = []
    for c in range(NCORES):
        xl = x[:, c * BL:(c + 1) * BL, :].reshape(t_steps * BL, DIN)
        xTl = np.ascontiguousarray(xl.T).astype(BF16)      # [DIN, rows]
        in_maps.append(
            dict(xt=xTl, wwt=wwT, ut=uT, biasr=biasr, identb=identb,
                 ones=ones)
        )
    return in_maps


def kernel(input_data, Ww, Wb, Uw, Ub, b, concatenate=1, _t_steps=None,
           _trace=False):
    x = np.asarray(input_data, dtype=np.float32)
    if _t_steps is not None:
        x = x[:_t_steps]
    Ww = np.asarray(Ww, dtype=np.float32)
    Uw = np.asarray(Uw, dtype=np.float32)
    bias = (
        np.asarray(Wb, dtype=np.float32)
        + np.asarray(Ub, dtype=np.float32)
        + np.asarray(b, dtype=np.float32)
    )

    t_steps = x.shape[0]
    if t_steps not in _NC_CACHE:
        _NC_CACHE[t_steps] = _build_nc(t_steps)
    nc = _NC_CACHE[t_steps]

    in_maps = _prep_inputs(x, Ww, Uw, bias, t_steps)

    global LAST_EXEC_NS, LAST_PROFILE
    res = run_bass_kernel_spmd(
        nc, in_maps, core_ids=list(range(NCORES)), trace=_trace
    )
    LAST_EXEC_NS = res.exec_time_ns
    LAST_PROFILE = res
    ys_full = np.concatenate(
        [
            np.asarray(res.results[c]["ys"], dtype=np.float32).reshape(
                t_steps, BL, DOUT
            )
            for c in range(NCORES)
        ],
        axis=1,
    )  # [T, B, DOUT]
    if concatenate:
        return ys_full.reshape(-1, DOUT)
    return ys_full
